# revision 48
# baseline (speedup 1.0000x reference)
"""Trainium2 Bass kernel: ResNet BasicBlock (conv3x3-BN-ReLU-mask-conv3x3-mask-BN-residual-ReLU).

Problem shape: x[4096, 64, 7, 7], both convs 64->64 3x3 pad 1.

Strategy (pure data parallel, 8 cores, 512 images/core):
  * Channels live on SBUF partitions. Two 64-channel image streams are
    stacked into the 128 partitions ("half0" -> partitions 0-63,
    "half1" -> 64-127) so elementwise engines run at full width.
  * A 3x3 conv is 9 shifted 64x64 matmuls accumulated in PSUM. No
    padding anywhere: each tap computes only its VALID output region
    (strided psum write, strided rhs window of the raw 7x7 tile). The
    center tap goes first with start=True covering the full region, the
    8 boundary taps accumulate partial regions. This skips the 18% of
    columns that a padded formulation spends multiplying zeros.
  * x is cast to bf16 on the host: the conv path used bf16 anyway, only
    the fp32 residual add sees the (tiny) rounding. Halves input DMA.
  * The 128x128 PE array is split into 4 64x64 quadrants via the matmul
    base partitions (rhs base -> row group, psum base -> column group).
    Four independent tap-chains (2 pairs x 2 halves) run concurrently.
  * BN scales are folded into the conv weights on the host; BN shifts
    are per-partition bias operands of the ScalarE/DVE finals.
  * Quad sizes ramp: tiny first quads so the first matmul starts as
    early as possible, tiny last quad so the conv2 tail is short.
  * The critic masks only touch batch element 0: every core runs the
    same mask multiply on its first image; cores 1-7 get all-ones masks.
"""

import ml_dtypes
import numpy as np

import concourse.bass as bass  # noqa: F401  (engine namespaces live on the nc object)
import concourse.tile as tile
from concourse import bacc, mybir
from concourse.bass_utils import run_bass_kernel_spmd

F32 = mybir.dt.float32
BF16 = mybir.dt.bfloat16
NP_BF16 = ml_dtypes.bfloat16
EPS = 1e-5
B, C, H, W = 4096, 64, 7, 7
HW = H * W
NCORES = 8
BPC = B // NCORES          # 512 images per core
SLOTS = BPC // 2           # 256 image slots per half-stream
NMAX = 10                  # max images per chain (PSUM bank: 490 of 512 fp32)

# Quad schedule: (base_slot, n images per chain). A quad = 4 concurrent
# chains of n images (4n images total). Small quads at the start (fast
# ramp: the first matmul only waits for a small DMA) and at the end
# (short conv2 drain tail); n=10 in the middle minimizes LDWEIGHTS tax.
_NSEQ = [3, 5] + [10] * 11 + [8, 2]
QUADS = []
_b = 0
for _n in _NSEQ:
    QUADS.append((_b, _n))
    _b += 2 * _n
assert sum(2 * n for _, n in QUADS) == SLOTS

# (pair_in_quad, half, colgroup): the 4 concurrent chains of a quad.
CHAINS = [(0, 0, 0), (1, 1, 0), (0, 1, 1), (1, 0, 1)]

# taps in emission order: center first (start=True covers the full
# [n,7,7] region), then the 8 boundary taps accumulate subregions.
TAPS = [4, 0, 1, 2, 3, 5, 6, 7, 8]


def _tap_geom(t):
    """For tap t=(kh*3+kw): output rows/cols [y0,y1)x[x0,x1) and the
    input window start (iy0, ix0). in[y+kh-1, x+kw-1], valid in 0..6."""
    kh, kw = t // 3, t % 3
    y0, y1 = max(0, 1 - kh), min(H, H + 1 - kh)
    x0, x1 = max(0, 1 - kw), min(W, W + 1 - kw)
    return y0, y1, x0, x1, y0 + kh - 1, x0 + kw - 1

_CACHE = {}


def _emit_conv(nc, src, w_sb, ps, n, eye_res=None):
    """One quad of one conv: 4 concurrent valid-region tap-chains.

    src: [128, >=2n, 49] bf16 tile (raw 7x7 images, no padding)
    ps:  [128, 2, 512] fp32 psum tile
    eye_res: optional (eye_sb, xin_q) - accumulate the identity residual
      on the PE as a 10th tap (keeps the drain-tail off the DVE)
    """
    last = 8 if eye_res is None else 9
    for idx, t in enumerate(TAPS):
        y0, y1, x0, x1, iy0, ix0 = _tap_geom(t)
        ny, nx = y1 - y0, x1 - x0
        for (j, half, cg) in CHAINS:
            rhs = src[64 * half:64 * half + 64,
                      n * j:n * (j + 1), 0:HW].rearrange(
                "p s (h w) -> p s h w", h=H, w=W)[
                :, :, iy0:iy0 + ny, ix0:ix0 + nx]
            lhsT = w_sb[64 * half:64 * half + 64, t, :]
            out = ps[64 * cg:64 * cg + 64, j, 0:n * HW].rearrange(
                "p (s h w) -> p s h w", s=n, h=H, w=W)[
                :, :, y0:y1, x0:x1]
            nc.tensor.matmul(out, lhsT, rhs,
                             start=(idx == 0), stop=(idx == last),
                             skip_group_check=(idx != 0))
    if eye_res is not None:
        # the pair-1 half-swap cancels over conv1+conv2, so psum block
        # cg of bank j holds the cg-aligned images: the residual rhs is
        # cg-aligned too (rides the diagonal quadrants, far enough
        # behind the start-tap that quadrant skew cannot reorder them)
        eye_sb, xin_q = eye_res
        for j in range(2):
            for cg in range(2):
                rhs = xin_q[64 * cg:64 * cg + 64, n * j:n * (j + 1), 0:HW]
                out = ps[64 * cg:64 * cg + 64, j, 0:n * HW]
                nc.tensor.matmul(out, eye_sb[64 * cg:64 * cg + 64, :], rhs,
                                 start=False, stop=True,
                                 skip_group_check=True)


def _psum_view(ps, j, n):
    """[128, n, 7, 7] view of pair j's bank of a [128, 2, 512] psum tile."""
    return ps[:, j, 0:n * HW].rearrange("p (i h w) -> p i h w", i=n, h=H, w=W)


def _build():
    nc = bacc.Bacc("TRN2", target_bir_lowering=False, debug=False,
                   num_devices=NCORES)
    x_d = nc.dram_tensor("x", [128, SLOTS, HW], BF16, kind="ExternalInput")
    w1_d = nc.dram_tensor("w1", [128, 9, 64], BF16, kind="ExternalInput")
    w2_d = nc.dram_tensor("w2", [128, 9, 64], BF16, kind="ExternalInput")
    eye_d = nc.dram_tensor("eye", [128, 64], BF16, kind="ExternalInput")
    cst_d = nc.dram_tensor("cst", [128, 2], F32, kind="ExternalInput")
    msk_d = nc.dram_tensor("msk", [64, 2, HW], F32, kind="ExternalInput")
    o_d = nc.dram_tensor("o", [128, SLOTS, HW], F32, kind="ExternalOutput")

    with tile.TileContext(nc) as tc:
        with (
            tc.tile_pool(name="singles", bufs=1) as singles,
            tc.tile_pool(name="xin", bufs=6) as xin_pool,
            tc.tile_pool(name="y1", bufs=3) as y1_pool,
            tc.tile_pool(name="outp", bufs=3) as out_pool,
            tc.tile_pool(name="ps1", bufs=2, space="PSUM") as ps1_pool,
            tc.tile_pool(name="ps2", bufs=2, space="PSUM") as ps2_pool,
        ):
            w1_sb = singles.tile([128, 9, 64], BF16, name="w1_sb")
            w2_sb = singles.tile([128, 9, 64], BF16, name="w2_sb")
            eye_sb = singles.tile([128, 64], BF16, name="eye_sb")
            cst_sb = singles.tile([128, 2], F32, name="cst_sb")
            msk_sb = singles.tile([64, 2, HW], F32, name="msk_sb")
            warm_sb = singles.tile([128, 1], F32, name="warm_sb")

            def emit_conv2(state):
                v, base, n, yp, xin_q = state
                tail = v >= len(QUADS) - 2
                ps2 = ps2_pool.tile([128, 2, 512], F32, name="ps2t")
                # drain-tail quads: the PE adds the identity residual as a
                # 10th tap, keeping the tail chain off the DVE
                _emit_conv(nc, yp, w2_sb, ps2, n,
                           eye_res=(eye_sb, xin_q) if tail else None)
                if v == 0:
                    # critic mask 2 on conv2 output of batch element 0
                    tgt = ps2[0:64, 0, 0:HW].rearrange(
                        "p (h w) -> p h w", h=H, w=W)
                    nc.vector.tensor_mul(tgt, tgt, msk_sb[:, 1, :].rearrange(
                        "p (h w) -> p h w", h=H, w=W))
                # residual adds in fp32 psum (x is bf16, error ~0.4% of |x|)
                views = [_psum_view(ps2, j, n) for j in range(2)]
                out_q = out_pool.tile([128, 2 * NMAX, HW], F32, name="out_q")
                if not tail:
                    for j in range(2):
                        nc.vector.tensor_add(
                            views[j], views[j],
                            xin_q[:, n * j:n * (j + 1), 0:HW].rearrange(
                                "p s (h w) -> p s h w", h=H, w=W))
                # the two relu(psum+shift2) finals split DVE/ACT so they
                # run concurrently
                nc.vector.tensor_scalar(
                    out_q[:, 0:n], views[0],
                    cst_sb[:, 1:2], 0.0,
                    mybir.AluOpType.add, mybir.AluOpType.max)
                nc.scalar.activation(
                    out=out_q[:, n:2 * n], in_=views[1],
                    func=mybir.ActivationFunctionType.Relu,
                    bias=cst_sb[:, 1:2], scale=1.0)
                if tail:
                    # stream each pair back the moment its final relu
                    # lands, on separate rings so the issues overlap
                    nc.sync.dma_start(o_d[:, base:base + n], out_q[:, 0:n])
                    nc.scalar.dma_start(o_d[:, base + n:base + 2 * n],
                                        out_q[:, n:2 * n])
                else:
                    nc.sync.dma_start(o_d[:, base:base + 2 * n],
                                      out_q[:, 0:2 * n])

            def emit_in_dma(v):
                base, n = QUADS[v]
                xin_q = xin_pool.tile([128, 2 * NMAX, HW], BF16, name="xin_q")
                src = x_d[:, base:base + 2 * n]
                # alternate input quads across the sync/ACT HWDGE rings:
                # halves each ring's byte load and issue serialization
                eng = nc.sync if v % 2 == 0 else nc.scalar
                eng.dma_start(xin_q[:, 0:2 * n], src)
                return xin_q

            pending = None
            xin_qs = {}
            for v, (base, n) in enumerate(QUADS):
                if v == 0:
                    # prologue critical path: w1 gates the first
                    # LDWEIGHTS, x quad 0 the first matmul. Split w1
                    # across both rings (64 partitions each) and put x0
                    # first on the sync ring so both land early.
                    nc.scalar.dma_start(w1_sb[0:64], w1_d[0:64])
                    xin_qs[0] = emit_in_dma(0)           # sync (v=0)
                    nc.sync.dma_start(w1_sb[64:128], w1_d[64:128])
                    nc.scalar.dma_start(cst_sb[:], cst_d[:])
                    xin_qs[1] = emit_in_dma(1)           # scalar (v=1)
                    nc.sync.dma_start(msk_sb[:], msk_d[:])
                    nc.scalar.dma_start(w2_sb[:], w2_d[:])
                    # eye (identity residual of the drain tail) is not
                    # needed until the end: the slow gpsimd ring is fine
                    nc.gpsimd.dma_start(eye_sb[:], eye_d[:])
                    # preload the ACT function table (contains Relu)
                    # behind the DMA issues on the ACT queue
                    nc.scalar.memzero(warm_sb[:])
                    xin_qs[2] = emit_in_dma(2)
                elif v + 2 < len(QUADS):
                    xin_qs[v + 2] = emit_in_dma(v + 2)
                xin_q = xin_qs.pop(v)
                ps1 = ps1_pool.tile([128, 2, 512], F32, name="ps1t")
                _emit_conv(nc, xin_q, w1_sb, ps1, n)
                yp = y1_pool.tile([128, 2 * NMAX, HW], BF16, name="y1_q")
                if v >= len(QUADS) - 2:
                    # drain tail: relu1 gates the tiny final conv2s with
                    # almost no PE work left to hide it - run the two
                    # pairs on DVE and ACT concurrently
                    nc.vector.tensor_scalar(
                        yp[:, 0:n].rearrange("p s w -> p (s w)"),
                        ps1[:, 0, 0:n * HW],
                        cst_sb[:, 0:1], 0.0,
                        mybir.AluOpType.add, mybir.AluOpType.max)
                    nc.scalar.activation(
                        out=yp[:, n:2 * n],
                        in_=_psum_view(ps1, 1, n),
                        func=mybir.ActivationFunctionType.Relu,
                        bias=cst_sb[:, 0:1], scale=1.0)
                else:
                    for j in range(2):
                        nc.scalar.activation(
                            out=yp[:, n * j:n * (j + 1)],
                            in_=_psum_view(ps1, j, n),
                            func=mybir.ActivationFunctionType.Relu,
                            bias=cst_sb[:, 0:1], scale=1.0)
                if v == 0:
                    # critic mask 1 on relu(bn1(conv1)) of batch elem 0
                    tgt = yp[0:64, 0, 0:HW].rearrange(
                        "p (h w) -> p h w", h=H, w=W)
                    nc.vector.tensor_mul(tgt, tgt, msk_sb[:, 0, :].rearrange(
                        "p (h w) -> p h w", h=H, w=W))
                if pending is not None:
                    emit_conv2(pending)
                pending = (v, base, n, yp, xin_q)
            emit_conv2(pending)

    nc.compile()
    return nc


def _get_nc():
    if "nc" not in _CACHE:
        _CACHE["nc"] = _build()
    return _CACHE["nc"]


def _host_pack(x, w1, g1, b1, m1, v1, w2, g2, b2, m2, v2, mask1, mask2):
    x = np.asarray(x, np.float32)
    scale1 = np.asarray(g1, np.float32) / np.sqrt(np.asarray(v1, np.float32) + EPS)
    shift1 = np.asarray(b1, np.float32) - np.asarray(m1, np.float32) * scale1
    scale2 = np.asarray(g2, np.float32) / np.sqrt(np.asarray(v2, np.float32) + EPS)
    shift2 = np.asarray(b2, np.float32) - np.asarray(m2, np.float32) * scale2

    def pack_w(w, scale):
        ws = np.asarray(w, np.float32) * scale[:, None, None, None]
        # [co, ci, kh, kw] -> [ci, tap, co], duplicated into both halves
        lhsT = ws.transpose(1, 2, 3, 0).reshape(64, 9, 64)
        return np.ascontiguousarray(np.tile(lhsT, (2, 1, 1)).astype(NP_BF16))

    wdev1, wdev2 = pack_w(w1, scale1), pack_w(w2, scale2)
    eye = np.ascontiguousarray(np.tile(np.eye(64), (2, 1)).astype(NP_BF16))
    cst = np.tile(np.stack([shift1, shift2], 1), (2, 1))
    cst = np.ascontiguousarray(cst.astype(np.float32))

    # image (v, j, half, i) -> batch idx 2*base + (2*j+half)*n + i,
    # device slot base + j*n + i, partition block half*64
    xb = x.reshape(NCORES, BPC, C, HW).astype(NP_BF16)
    xdev = np.empty((NCORES, 128, SLOTS, HW), NP_BF16)
    for base, n in QUADS:
        for j in range(2):
            for h in range(2):
                lo = 2 * base + (2 * j + h) * n
                xdev[:, 64 * h:64 * h + 64, base + j * n:base + (j + 1) * n] = \
                    xb[:, lo:lo + n].transpose(0, 2, 1, 3)
    xdev = np.ascontiguousarray(xdev)

    msk0 = np.ascontiguousarray(np.stack(
        [np.asarray(mask1, np.float32).reshape(C, HW),
         np.asarray(mask2, np.float32).reshape(C, HW)], 1))
    msk1s = np.ones_like(msk0)

    in_maps = []
    for c in range(NCORES):
        in_maps.append({
            "x": xdev[c],
            "w1": wdev1,
            "w2": wdev2,
            "eye": eye,
            "cst": cst,
            "msk": msk0 if c == 0 else msk1s,
        })
    return in_maps


def _host_unpack(results):
    o = np.stack([results[c]["o"] for c in range(NCORES)])
    out = np.empty((NCORES, BPC, C, HW), np.float32)
    for base, n in QUADS:
        for j in range(2):
            for h in range(2):
                lo = 2 * base + (2 * j + h) * n
                out[:, lo:lo + n] = o[
                    :, 64 * h:64 * h + 64,
                    base + j * n:base + (j + 1) * n].transpose(0, 2, 1, 3)
    return np.ascontiguousarray(out.reshape(B, C, H, W))


def run(trace=False, **inputs):
    nc = _get_nc()
    in_maps = _host_pack(**inputs)
    res = run_bass_kernel_spmd(nc, in_maps, core_ids=list(range(NCORES)),
                               trace=trace)
    return _host_unpack(res.results), res


def kernel(**inputs) -> np.ndarray:
    out, _ = run(trace=False, **inputs)
    return out


# revision 49
# speedup vs baseline: 1.0002x; 1.0002x over previous
"""Trainium2 Bass kernel: ResNet BasicBlock (conv3x3-BN-ReLU-mask-conv3x3-mask-BN-residual-ReLU).

Problem shape: x[4096, 64, 7, 7], both convs 64->64 3x3 pad 1.

Strategy (pure data parallel, 8 cores, 512 images/core):
  * Channels live on SBUF partitions. Two 64-channel image streams are
    stacked into the 128 partitions ("half0" -> partitions 0-63,
    "half1" -> 64-127) so elementwise engines run at full width.
  * A 3x3 conv is 9 shifted 64x64 matmuls accumulated in PSUM. No
    padding anywhere: each tap computes only its VALID output region
    (strided psum write, strided rhs window of the raw 7x7 tile). The
    center tap goes first with start=True covering the full region, the
    8 boundary taps accumulate partial regions. This skips the 18% of
    columns that a padded formulation spends multiplying zeros.
  * x is cast to bf16 on the host: the conv path used bf16 anyway, only
    the fp32 residual add sees the (tiny) rounding. Halves input DMA.
  * The 128x128 PE array is split into 4 64x64 quadrants via the matmul
    base partitions (rhs base -> row group, psum base -> column group).
    Four independent tap-chains (2 pairs x 2 halves) run concurrently.
  * BN scales are folded into the conv weights on the host; BN shifts
    are per-partition bias operands of the ScalarE/DVE finals.
  * Quad sizes ramp: tiny first quads so the first matmul starts as
    early as possible, tiny last quad so the conv2 tail is short.
  * The critic masks only touch batch element 0: every core runs the
    same mask multiply on its first image; cores 1-7 get all-ones masks.
"""

import ml_dtypes
import numpy as np

import concourse.bass as bass  # noqa: F401  (engine namespaces live on the nc object)
import concourse.tile as tile
from concourse import bacc, mybir
from concourse.bass_utils import run_bass_kernel_spmd

F32 = mybir.dt.float32
BF16 = mybir.dt.bfloat16
NP_BF16 = ml_dtypes.bfloat16
EPS = 1e-5
B, C, H, W = 4096, 64, 7, 7
HW = H * W
NCORES = 8
BPC = B // NCORES          # 512 images per core
SLOTS = BPC // 2           # 256 image slots per half-stream
NMAX = 10                  # max images per chain (PSUM bank: 490 of 512 fp32)

# Quad schedule: (base_slot, n images per chain). A quad = 4 concurrent
# chains of n images (4n images total). Small quads at the start (fast
# ramp: the first matmul only waits for a small DMA) and at the end
# (short conv2 drain tail); n=10 in the middle minimizes LDWEIGHTS tax.
_NSEQ = [3, 6] + [10] * 11 + [7, 2]
QUADS = []
_b = 0
for _n in _NSEQ:
    QUADS.append((_b, _n))
    _b += 2 * _n
assert sum(2 * n for _, n in QUADS) == SLOTS

# (pair_in_quad, half, colgroup): the 4 concurrent chains of a quad.
CHAINS = [(0, 0, 0), (1, 1, 0), (0, 1, 1), (1, 0, 1)]

# taps in emission order: center first (start=True covers the full
# [n,7,7] region), then the 8 boundary taps accumulate subregions.
TAPS = [4, 0, 1, 2, 3, 5, 6, 7, 8]


def _tap_geom(t):
    """For tap t=(kh*3+kw): output rows/cols [y0,y1)x[x0,x1) and the
    input window start (iy0, ix0). in[y+kh-1, x+kw-1], valid in 0..6."""
    kh, kw = t // 3, t % 3
    y0, y1 = max(0, 1 - kh), min(H, H + 1 - kh)
    x0, x1 = max(0, 1 - kw), min(W, W + 1 - kw)
    return y0, y1, x0, x1, y0 + kh - 1, x0 + kw - 1

_CACHE = {}


def _emit_conv(nc, src, w_sb, ps, n, eye_res=None):
    """One quad of one conv: 4 concurrent valid-region tap-chains.

    src: [128, >=2n, 49] bf16 tile (raw 7x7 images, no padding)
    ps:  [128, 2, 512] fp32 psum tile
    eye_res: optional (eye_sb, xin_q) - accumulate the identity residual
      on the PE as a 10th tap (keeps the drain-tail off the DVE)
    """
    last = 8 if eye_res is None else 9
    for idx, t in enumerate(TAPS):
        y0, y1, x0, x1, iy0, ix0 = _tap_geom(t)
        ny, nx = y1 - y0, x1 - x0
        for (j, half, cg) in CHAINS:
            rhs = src[64 * half:64 * half + 64,
                      n * j:n * (j + 1), 0:HW].rearrange(
                "p s (h w) -> p s h w", h=H, w=W)[
                :, :, iy0:iy0 + ny, ix0:ix0 + nx]
            lhsT = w_sb[64 * half:64 * half + 64, t, :]
            out = ps[64 * cg:64 * cg + 64, j, 0:n * HW].rearrange(
                "p (s h w) -> p s h w", s=n, h=H, w=W)[
                :, :, y0:y1, x0:x1]
            nc.tensor.matmul(out, lhsT, rhs,
                             start=(idx == 0), stop=(idx == last),
                             skip_group_check=(idx != 0))
    if eye_res is not None:
        # the pair-1 half-swap cancels over conv1+conv2, so psum block
        # cg of bank j holds the cg-aligned images: the residual rhs is
        # cg-aligned too (rides the diagonal quadrants, far enough
        # behind the start-tap that quadrant skew cannot reorder them)
        eye_sb, xin_q = eye_res
        for j in range(2):
            for cg in range(2):
                rhs = xin_q[64 * cg:64 * cg + 64, n * j:n * (j + 1), 0:HW]
                out = ps[64 * cg:64 * cg + 64, j, 0:n * HW]
                nc.tensor.matmul(out, eye_sb[64 * cg:64 * cg + 64, :], rhs,
                                 start=False, stop=True,
                                 skip_group_check=True)


def _psum_view(ps, j, n):
    """[128, n, 7, 7] view of pair j's bank of a [128, 2, 512] psum tile."""
    return ps[:, j, 0:n * HW].rearrange("p (i h w) -> p i h w", i=n, h=H, w=W)


def _build():
    nc = bacc.Bacc("TRN2", target_bir_lowering=False, debug=False,
                   num_devices=NCORES)
    x_d = nc.dram_tensor("x", [128, SLOTS, HW], BF16, kind="ExternalInput")
    w1_d = nc.dram_tensor("w1", [128, 9, 64], BF16, kind="ExternalInput")
    w2_d = nc.dram_tensor("w2", [128, 9, 64], BF16, kind="ExternalInput")
    eye_d = nc.dram_tensor("eye", [128, 64], BF16, kind="ExternalInput")
    cst_d = nc.dram_tensor("cst", [128, 2], F32, kind="ExternalInput")
    msk_d = nc.dram_tensor("msk", [64, 2, HW], F32, kind="ExternalInput")
    o_d = nc.dram_tensor("o", [128, SLOTS, HW], F32, kind="ExternalOutput")

    with tile.TileContext(nc) as tc:
        with (
            tc.tile_pool(name="singles", bufs=1) as singles,
            tc.tile_pool(name="xin", bufs=6) as xin_pool,
            tc.tile_pool(name="y1", bufs=3) as y1_pool,
            tc.tile_pool(name="outp", bufs=3) as out_pool,
            tc.tile_pool(name="ps1", bufs=2, space="PSUM") as ps1_pool,
            tc.tile_pool(name="ps2", bufs=2, space="PSUM") as ps2_pool,
        ):
            w1_sb = singles.tile([128, 9, 64], BF16, name="w1_sb")
            w2_sb = singles.tile([128, 9, 64], BF16, name="w2_sb")
            eye_sb = singles.tile([128, 64], BF16, name="eye_sb")
            cst_sb = singles.tile([128, 2], F32, name="cst_sb")
            msk_sb = singles.tile([64, 2, HW], F32, name="msk_sb")
            warm_sb = singles.tile([128, 1], F32, name="warm_sb")

            def emit_conv2(state):
                v, base, n, yp, xin_q = state
                tail = v >= len(QUADS) - 2
                ps2 = ps2_pool.tile([128, 2, 512], F32, name="ps2t")
                # drain-tail quads: the PE adds the identity residual as a
                # 10th tap, keeping the tail chain off the DVE
                _emit_conv(nc, yp, w2_sb, ps2, n,
                           eye_res=(eye_sb, xin_q) if tail else None)
                if v == 0:
                    # critic mask 2 on conv2 output of batch element 0
                    tgt = ps2[0:64, 0, 0:HW].rearrange(
                        "p (h w) -> p h w", h=H, w=W)
                    nc.vector.tensor_mul(tgt, tgt, msk_sb[:, 1, :].rearrange(
                        "p (h w) -> p h w", h=H, w=W))
                # residual adds in fp32 psum (x is bf16, error ~0.4% of |x|)
                views = [_psum_view(ps2, j, n) for j in range(2)]
                out_q = out_pool.tile([128, 2 * NMAX, HW], F32, name="out_q")
                if not tail:
                    for j in range(2):
                        nc.vector.tensor_add(
                            views[j], views[j],
                            xin_q[:, n * j:n * (j + 1), 0:HW].rearrange(
                                "p s (h w) -> p s h w", h=H, w=W))
                # the two relu(psum+shift2) finals split DVE/ACT so they
                # run concurrently
                nc.vector.tensor_scalar(
                    out_q[:, 0:n], views[0],
                    cst_sb[:, 1:2], 0.0,
                    mybir.AluOpType.add, mybir.AluOpType.max)
                nc.scalar.activation(
                    out=out_q[:, n:2 * n], in_=views[1],
                    func=mybir.ActivationFunctionType.Relu,
                    bias=cst_sb[:, 1:2], scale=1.0)
                if tail:
                    # stream each pair back the moment its final relu
                    # lands, on separate rings so the issues overlap
                    nc.sync.dma_start(o_d[:, base:base + n], out_q[:, 0:n])
                    nc.scalar.dma_start(o_d[:, base + n:base + 2 * n],
                                        out_q[:, n:2 * n])
                else:
                    nc.sync.dma_start(o_d[:, base:base + 2 * n],
                                      out_q[:, 0:2 * n])

            def emit_in_dma(v):
                base, n = QUADS[v]
                xin_q = xin_pool.tile([128, 2 * NMAX, HW], BF16, name="xin_q")
                src = x_d[:, base:base + 2 * n]
                # alternate input quads across the sync/ACT HWDGE rings:
                # halves each ring's byte load and issue serialization
                eng = nc.sync if v % 2 == 0 else nc.scalar
                eng.dma_start(xin_q[:, 0:2 * n], src)
                return xin_q

            pending = None
            xin_qs = {}
            for v, (base, n) in enumerate(QUADS):
                if v == 0:
                    # prologue critical path: w1 gates the first
                    # LDWEIGHTS, x quad 0 the first matmul. Split w1
                    # across both rings (64 partitions each) and put x0
                    # first on the sync ring so both land early.
                    nc.scalar.dma_start(w1_sb[0:64], w1_d[0:64])
                    xin_qs[0] = emit_in_dma(0)           # sync (v=0)
                    nc.sync.dma_start(w1_sb[64:128], w1_d[64:128])
                    nc.scalar.dma_start(cst_sb[:], cst_d[:])
                    xin_qs[1] = emit_in_dma(1)           # scalar (v=1)
                    nc.sync.dma_start(msk_sb[:], msk_d[:])
                    nc.scalar.dma_start(w2_sb[:], w2_d[:])
                    # eye (identity residual of the drain tail) is not
                    # needed until the end: the slow gpsimd ring is fine
                    nc.gpsimd.dma_start(eye_sb[:], eye_d[:])
                    # preload the ACT function table (contains Relu)
                    # behind the DMA issues on the ACT queue
                    nc.scalar.memzero(warm_sb[:])
                    xin_qs[2] = emit_in_dma(2)
                elif v + 2 < len(QUADS):
                    xin_qs[v + 2] = emit_in_dma(v + 2)
                xin_q = xin_qs.pop(v)
                ps1 = ps1_pool.tile([128, 2, 512], F32, name="ps1t")
                _emit_conv(nc, xin_q, w1_sb, ps1, n)
                yp = y1_pool.tile([128, 2 * NMAX, HW], BF16, name="y1_q")
                if v >= len(QUADS) - 2:
                    # drain tail: relu1 gates the tiny final conv2s with
                    # almost no PE work left to hide it - run the two
                    # pairs on DVE and ACT concurrently
                    nc.vector.tensor_scalar(
                        yp[:, 0:n].rearrange("p s w -> p (s w)"),
                        ps1[:, 0, 0:n * HW],
                        cst_sb[:, 0:1], 0.0,
                        mybir.AluOpType.add, mybir.AluOpType.max)
                    nc.scalar.activation(
                        out=yp[:, n:2 * n],
                        in_=_psum_view(ps1, 1, n),
                        func=mybir.ActivationFunctionType.Relu,
                        bias=cst_sb[:, 0:1], scale=1.0)
                else:
                    for j in range(2):
                        nc.scalar.activation(
                            out=yp[:, n * j:n * (j + 1)],
                            in_=_psum_view(ps1, j, n),
                            func=mybir.ActivationFunctionType.Relu,
                            bias=cst_sb[:, 0:1], scale=1.0)
                if v == 0:
                    # critic mask 1 on relu(bn1(conv1)) of batch elem 0
                    tgt = yp[0:64, 0, 0:HW].rearrange(
                        "p (h w) -> p h w", h=H, w=W)
                    nc.vector.tensor_mul(tgt, tgt, msk_sb[:, 0, :].rearrange(
                        "p (h w) -> p h w", h=H, w=W))
                if pending is not None:
                    emit_conv2(pending)
                pending = (v, base, n, yp, xin_q)
            emit_conv2(pending)

    nc.compile()
    return nc


def _get_nc():
    if "nc" not in _CACHE:
        _CACHE["nc"] = _build()
    return _CACHE["nc"]


def _host_pack(x, w1, g1, b1, m1, v1, w2, g2, b2, m2, v2, mask1, mask2):
    x = np.asarray(x, np.float32)
    scale1 = np.asarray(g1, np.float32) / np.sqrt(np.asarray(v1, np.float32) + EPS)
    shift1 = np.asarray(b1, np.float32) - np.asarray(m1, np.float32) * scale1
    scale2 = np.asarray(g2, np.float32) / np.sqrt(np.asarray(v2, np.float32) + EPS)
    shift2 = np.asarray(b2, np.float32) - np.asarray(m2, np.float32) * scale2

    def pack_w(w, scale):
        ws = np.asarray(w, np.float32) * scale[:, None, None, None]
        # [co, ci, kh, kw] -> [ci, tap, co], duplicated into both halves
        lhsT = ws.transpose(1, 2, 3, 0).reshape(64, 9, 64)
        return np.ascontiguousarray(np.tile(lhsT, (2, 1, 1)).astype(NP_BF16))

    wdev1, wdev2 = pack_w(w1, scale1), pack_w(w2, scale2)
    eye = np.ascontiguousarray(np.tile(np.eye(64), (2, 1)).astype(NP_BF16))
    cst = np.tile(np.stack([shift1, shift2], 1), (2, 1))
    cst = np.ascontiguousarray(cst.astype(np.float32))

    # image (v, j, half, i) -> batch idx 2*base + (2*j+half)*n + i,
    # device slot base + j*n + i, partition block half*64
    xb = x.reshape(NCORES, BPC, C, HW).astype(NP_BF16)
    xdev = np.empty((NCORES, 128, SLOTS, HW), NP_BF16)
    for base, n in QUADS:
        for j in range(2):
            for h in range(2):
                lo = 2 * base + (2 * j + h) * n
                xdev[:, 64 * h:64 * h + 64, base + j * n:base + (j + 1) * n] = \
                    xb[:, lo:lo + n].transpose(0, 2, 1, 3)
    xdev = np.ascontiguousarray(xdev)

    msk0 = np.ascontiguousarray(np.stack(
        [np.asarray(mask1, np.float32).reshape(C, HW),
         np.asarray(mask2, np.float32).reshape(C, HW)], 1))
    msk1s = np.ones_like(msk0)

    in_maps = []
    for c in range(NCORES):
        in_maps.append({
            "x": xdev[c],
            "w1": wdev1,
            "w2": wdev2,
            "eye": eye,
            "cst": cst,
            "msk": msk0 if c == 0 else msk1s,
        })
    return in_maps


def _host_unpack(results):
    o = np.stack([results[c]["o"] for c in range(NCORES)])
    out = np.empty((NCORES, BPC, C, HW), np.float32)
    for base, n in QUADS:
        for j in range(2):
            for h in range(2):
                lo = 2 * base + (2 * j + h) * n
                out[:, lo:lo + n] = o[
                    :, 64 * h:64 * h + 64,
                    base + j * n:base + (j + 1) * n].transpose(0, 2, 1, 3)
    return np.ascontiguousarray(out.reshape(B, C, H, W))


def run(trace=False, **inputs):
    nc = _get_nc()
    in_maps = _host_pack(**inputs)
    res = run_bass_kernel_spmd(nc, in_maps, core_ids=list(range(NCORES)),
                               trace=trace)
    return _host_unpack(res.results), res


def kernel(**inputs) -> np.ndarray:
    out, _ = run(trace=False, **inputs)
    return out
